# revision 25
# baseline (speedup 1.0000x reference)
"""Cross_Atten_Lite_split Trainium2 Bass kernel.

Sharding: 8 cores = (batch b in 0..3) x (query-half qh in 0..1).
Each core computes both attention heads for 2048 queries x 4096 keys of
its batch. No collectives. Math rewrites (validated vs reference):
  - eval-mode BN on x1/x2 folded into kq1_w/kq2_w (+bias).
  - channel_shuffle is a permutation of the shared q/k contraction axis
    -> eliminated;  k_h = [kq1[:,64h:64h+32]; kq2[:,64h:64h+32]],
    q_h likewise from rows 64h+32:64h+64.
  - K bias cancels in softmax (adds a per-query-row constant); dropped.
  - final BN + w_scale folded into out_w/out_b.
  - softmax without max-subtraction (max |score| ~ 67.5 < 88, fp32 safe).
  - softmax denominator via ones-augmented V (row 64 of PV output).
Matmuls run as float32r (fp32 bits, full-speed PE path at N=512).

Built on bacc.Bacc + nc.compile(): generate_event_semaphores splits
multi-wait instructions to satisfy the TRN2 1-wait-per-instruction
constraint.
"""

import numpy as np
from contextlib import ExitStack

import concourse.bass as bass
import concourse.bacc as bacc
import concourse.mybir as mybir
import concourse.tile as tile
from concourse.bass_utils import run_bass_kernel_spmd
from concourse.masks import make_identity

F32 = mybir.dt.float32
F32R = mybir.dt.float32r
AF = mybir.ActivationFunctionType

C = 256          # channels (INC1 == INC2)
N = 4096         # tokens per batch (64*64)
NQ = 2048        # queries per core
NT = 512         # free-dim tile size


def build_bass():
    nc = bacc.Bacc("TRN2", target_bir_lowering=False, debug=False, num_devices=8)

    x1T = nc.dram_tensor("x1T", [C, N], F32R, kind="ExternalInput").ap()
    x2T = nc.dram_tensor("x2T", [C, N], F32R, kind="ExternalInput").ap()
    xT = nc.dram_tensor("xT", [C, N], F32R, kind="ExternalInput").ap()
    wkq1 = nc.dram_tensor("wkq1", [2, 128, 128], F32R, kind="ExternalInput").ap()
    wkq2 = nc.dram_tensor("wkq2", [2, 128, 128], F32R, kind="ExternalInput").ap()
    wv = nc.dram_tensor("wv", [2, 128, 128], F32R, kind="ExternalInput").ap()
    wout = nc.dram_tensor("wout", [128, 256], F32R, kind="ExternalInput").ap()
    bq = nc.dram_tensor("bq", [128, 1], F32, kind="ExternalInput").ap()
    bv = nc.dram_tensor("bv", [128, 1], F32, kind="ExternalInput").ap()
    bout = nc.dram_tensor("bout", [2, 128, 1], F32, kind="ExternalInput").ap()
    onesd = nc.dram_tensor("onesd", [128, 64], F32R, kind="ExternalInput").ap()
    outT = nc.dram_tensor("outT", [C, NQ], F32, kind="ExternalOutput").ap()

    with ExitStack() as ctx:
        tc = ctx.enter_context(tile.TileContext(nc))
        const = ctx.enter_context(tc.tile_pool(name="const", bufs=1))
        pers = ctx.enter_context(tc.tile_pool(name="pers", bufs=1))

        # constants
        w_kq1 = [const.tile([128, 128], F32R, name=f"wkq1_{g}") for g in range(2)]
        w_kq2 = [const.tile([128, 128], F32R, name=f"wkq2_{g}") for g in range(2)]
        w_v = [const.tile([128, 128], F32R, name=f"wv_{g}") for g in range(2)]
        w_out = const.tile([128, 256], F32R, name="wout")
        b_q = const.tile([128, 1], F32, name="bq")
        b_v = const.tile([128, 1], F32, name="bv")
        b_out = [const.tile([128, 1], F32, name=f"bout_{g}") for g in range(2)]
        ident = const.tile([128, 128], F32, name="ident")
        ones1 = const.tile([1, 64], F32R, name="ones1")



        for g in range(2):
            nc.sync.dma_start(out=w_kq1[g][:], in_=wkq1[g])
            nc.sync.dma_start(out=w_kq2[g][:], in_=wkq2[g])
            nc.sync.dma_start(out=w_v[g][:], in_=wv[g])
            nc.sync.dma_start(out=b_out[g][:], in_=bout[g])
        nc.sync.dma_start(out=w_out[:], in_=wout[:])
        nc.sync.dma_start(out=b_q[:], in_=bq[:])
        nc.sync.dma_start(out=b_v[:], in_=bv[:])
        make_identity(nc, ident[:])
        nc.sync.dma_start(out=ones1[:], in_=onesd[0:1, 0:64])

        # persistent SBUF
        KT = pers.tile([128, N], F32R, name="KT")      # rows k1a,k1b,k2a,k2b
        QT = pers.tile([128, NQ], F32R, name="QT")     # rows q1a,q1b,q2a,q2b
        Vtok = pers.tile([128, 32 * 130], F32R, name="Vtok")
        xsb = [pers.tile([128, N], F32R, name=f"xsb_{g}") for g in range(2)]
        x1sb = [pers.tile([128, N], F32R, name=f"x1sb_{g}") for g in range(2)]
        x2sb = [pers.tile([128, N], F32R, name=f"x2sb_{g}") for g in range(2)]
        Ocat = pers.tile([128, NQ], F32R, name="Ocat")

        # fill the two ones-columns of each Vtok m-block via strided DMA
        vtok3 = Vtok.rearrange("p (m c) -> p m c", c=130)
        nc.sync.dma_start(out=vtok3[:, :, 64:65], in_=onesd[:, 0:32].rearrange("p (m c) -> p m c", c=1))
        nc.sync.dma_start(out=vtok3[:, :, 129:130], in_=onesd[:, 32:64].rearrange("p (m c) -> p m c", c=1))
        # DVE pre-touch of bias consts so later DVE ops don't wait on DMA queues
        btch = const.tile([128, 4], F32, name="btch")
        nc.vector.tensor_copy(btch[:, 0:1], b_v[:])
        nc.vector.tensor_copy(btch[:, 1:2], b_q[:])
        nc.vector.tensor_copy(btch[:, 2:3], b_out[0][:])
        nc.vector.tensor_copy(btch[:, 3:4], b_out[1][:])
        for g in range(2):
            for t in range(8):
                cs = slice(t * NT, (t + 1) * NT)
                rs = slice(128 * g, 128 * (g + 1))
                nc.sync.dma_start(out=xsb[g][:, cs], in_=xT[rs, cs])
                nc.sync.dma_start(out=x1sb[g][:, cs], in_=x1T[rs, cs])
                nc.sync.dma_start(out=x2sb[g][:, cs], in_=x2T[rs, cs])

        poolE = ctx.enter_context(tc.tile_pool(name="poolE", bufs=2))
        small = ctx.enter_context(tc.tile_pool(name="small", bufs=2))
        pout = ctx.enter_context(tc.tile_pool(name="pout", bufs=8))
        # ---------- Phase A: projections ----------
        with ExitStack() as actx:
            pvt = actx.enter_context(tc.tile_pool(name="pvt", bufs=2))
            poolA = actx.enter_context(tc.tile_pool(name="poolA", bufs=3, space="PSUM"))
            poolT = actx.enter_context(tc.tile_pool(name="poolT", bufs=4, space="PSUM"))

            for t in range(8):
                cs = slice(t * NT, (t + 1) * NT)
                if t >= 1:
                    # DVE sem is monotone: joining on iter t-1's last DVE
                    # write covers every DVE dep from iters <= t-1
                    mprev = 4 * (t - 1) + 3
                kq1p = poolA.tile([128, NT], F32, tag="mmA", name=f"kq1p_{t}")
                nc.tensor.matmul(kq1p[:], w_kq1[0][:], x1sb[0][:, cs], start=True, stop=False)
                nc.tensor.matmul(kq1p[:], w_kq1[1][:], x1sb[1][:, cs], start=False, stop=True)
                kq2p = poolA.tile([128, NT], F32, tag="mmA", name=f"kq2p_{t}")
                nc.tensor.matmul(kq2p[:], w_kq2[0][:], x2sb[0][:, cs], start=True, stop=False)
                nc.tensor.matmul(kq2p[:], w_kq2[1][:], x2sb[1][:, cs], start=False, stop=True)
                vp = poolA.tile([128, NT], F32, tag="mmA", name=f"vp_{t}")
                nc.tensor.matmul(vp[:], w_v[0][:], xsb[0][:, cs], start=True, stop=False)
                nc.tensor.matmul(vp[:], w_v[1][:], xsb[1][:, cs], start=False, stop=True)

                # scatter K/Q rows straight from PSUM (DVE only -> single sem);
                # Q bias applied during the scatter (tensor_scalar_add)
                nc.vector.tensor_copy(KT[0:32, cs], kq1p[0:32, :])
                nc.vector.tensor_copy(KT[32:64, cs], kq2p[0:32, :])
                nc.vector.tensor_copy(KT[64:96, cs], kq1p[64:96, :])
                nc.vector.tensor_copy(KT[96:128, cs], kq2p[64:96, :])
                if t < 4:  # query half
                    nc.vector.tensor_scalar_add(QT[0:32, cs], kq1p[32:64, :], b_q[0:32, :])
                    nc.vector.tensor_scalar_add(QT[32:64, cs], kq2p[32:64, :], b_q[32:64, :])
                    nc.vector.tensor_scalar_add(QT[64:96, cs], kq1p[96:128, :], b_q[64:96, :])
                    nc.vector.tensor_scalar_add(QT[96:128, cs], kq2p[96:128, :], b_q[96:128, :])
                VT = pvt.tile([128, NT], F32, tag="VT", name=f"VT_{t}")
                nc.vector.tensor_scalar_add(VT[:], vp[:], b_v[:])

                # transpose V for PV matmuls: Vtok[m] cols 0:64 = v1, 65:129 = v2
                for s in range(4):
                    m = 4 * t + s
                    ms = slice(s * 128, (s + 1) * 128)
                    tp = poolT.tile([128, 128], F32, tag="tp", name=f"tp_{m}")
                    nc.tensor.transpose(tp[:], VT[:, ms], ident[:])
                    nc.vector.tensor_copy(Vtok[:, m * 130:m * 130 + 64], tp[:, 0:64])
                    nc.vector.tensor_copy(Vtok[:, m * 130 + 65:m * 130 + 129], tp[:, 64:128])


        # ---------- Phase B: attention ----------
        with ExitStack() as bctx:
            poolS = bctx.enter_context(tc.tile_pool(name="poolS", bufs=1, space="PSUM"))
            poolO = bctx.enter_context(tc.tile_pool(name="poolO", bufs=2, space="PSUM"))
            poolCp = bctx.enter_context(tc.tile_pool(name="poolCp", bufs=1, space="PSUM"))

            for h in range(2):
                hs = slice(64 * h, 64 * (h + 1))
                for j in range(4):
                    qs = slice(j * NT, (j + 1) * NT)
                    op = poolO.tile([65, NT], F32, tag="op", name=f"op_{h}_{j}")
                    for blk in range(8):
                        sp = poolS.tile([128, 4 * NT], F32, tag="sp", name=f"sp_{h}_{j}_{blk}")
                        for b4 in range(4):
                            m = 4 * blk + b4
                            nc.tensor.matmul(
                                sp[:, b4 * NT:(b4 + 1) * NT],
                                KT[hs, m * 128:(m + 1) * 128],
                                QT[hs, qs],
                                start=True, stop=True)
                        et = poolE.tile([128, 4 * NT], F32R, tag="et", name=f"et_{h}_{j}_{blk}")
                        nc.scalar.activation(et[:], sp[:], AF.Exp, scale=0.125)
                        for b4 in range(4):
                            m = 4 * blk + b4
                            nc.tensor.matmul(
                                op[:],
                                Vtok[:, m * 130 + 65 * h:m * 130 + 65 * h + 65],
                                et[:, b4 * NT:(b4 + 1) * NT],
                                start=(blk == 0 and b4 == 0),
                                stop=(blk == 7 and b4 == 3))

                    # normalize: o[d, n] * (1 / denom[n]); denom = op[64, :]
                    rec = small.tile([1, NT], F32R, tag="rec", name=f"rec_{h}_{j}")
                    with nc.allow_low_precision(reason="f32r is fp32 bits"):
                        nc.vector.reciprocal(rec[:], op[64:65, :])
                    rb = poolCp.tile([64, NT], F32, tag="rb", name=f"rb_{h}_{j}")
                    nc.tensor.matmul(rb[:], ones1[:], rec[:], start=True, stop=True)
                    rbs = small.tile([64, NT], F32, tag="rbs", name=f"rbs_{h}_{j}")
                    nc.vector.tensor_copy(rbs[:], rb[:])
                    nc.vector.tensor_mul(Ocat[hs, qs], op[0:64, :], rbs[:])

            # ---------- Phase C: output projection + bias + residual ----------
            for j in range(4):
                qs = slice(j * NT, (j + 1) * NT)
                for g in range(2):
                    pp = poolCp.tile([128, NT], F32, tag="rb", name=f"pp_{j}_{g}")
                    nc.tensor.matmul(pp[:], w_out[:, 128 * g:128 * (g + 1)],
                                     Ocat[:, qs], start=True, stop=True)
                    osb = pout.tile([128, NT], F32, tag="osb", name=f"osb_{j}_{g}")
                    nc.vector.scalar_tensor_tensor(
                        osb[:], pp[:], b_out[g][:], xsb[g][:, qs].bitcast(F32),
                        op0=bass.mybir.AluOpType.add, op1=bass.mybir.AluOpType.add)
                    nc.sync.dma_start(out=outT[128 * g:128 * (g + 1), qs], in_=osb[:])

    nc.compile()
    return nc


_NC = None


def _get_nc():
    global _NC
    if _NC is None:
        _NC = build_bass()
    return _NC


def kernel(**inputs):
    out, _ = _run(inputs, trace=False)
    return out


def _run(inputs, trace=False):
    eps = 1e-5
    f32 = np.float32
    inp = {k: np.asarray(v, dtype=np.float32) for k, v in inputs.items()}

    s1 = inp['bn1_g'] / np.sqrt(inp['bn1_v'] + eps)
    t1 = inp['bn1_b'] - inp['bn1_m'] * s1
    s2 = inp['bn2_g'] / np.sqrt(inp['bn2_v'] + eps)
    t2 = inp['bn2_b'] - inp['bn2_m'] * s2
    W1 = inp['kq1_w'] * s1[None, :]
    b1 = inp['kq1_b'] + inp['kq1_w'] @ t1
    W2 = inp['kq2_w'] * s2[None, :]
    b2 = inp['kq2_b'] + inp['kq2_w'] @ t2
    sl = inp['bnl_g'] / np.sqrt(inp['bnl_v'] + eps)
    tl = inp['bnl_b'] - inp['bnl_m'] * sl
    ws = inp['w_scale'][0]
    Wout = (ws * sl)[:, None] * inp['out_w']
    bout_f = ws * (sl * inp['out_b'] + tl)

    wkq1 = np.ascontiguousarray(W1.T.reshape(2, 128, 128), dtype=f32)
    wkq2 = np.ascontiguousarray(W2.T.reshape(2, 128, 128), dtype=f32)
    wv = np.ascontiguousarray(inp['v_w'].T.reshape(2, 128, 128), dtype=f32)
    wout_a = np.ascontiguousarray(Wout.T, dtype=f32)
    bq = np.concatenate([b1[32:64], b2[32:64], b1[96:128], b2[96:128]]
                        ).reshape(128, 1).astype(f32)
    bv = inp['v_b'].reshape(128, 1).astype(f32)
    bout_a = bout_f.reshape(2, 128, 1).astype(f32)

    shared = dict(wkq1=wkq1, wkq2=wkq2, wv=wv, wout=wout_a, bq=bq, bv=bv,
                  bout=bout_a, onesd=np.ones((128, 64), dtype=f32))

    in_maps = []
    for b in range(4):
        x1Tb = inp['x1'][b].reshape(C, N)
        x2Tb = inp['x2'][b].reshape(C, N)
        xTb = inp['x'][b].reshape(C, N)
        for qh in range(2):
            if qh == 0:
                m = dict(x1T=np.ascontiguousarray(x1Tb),
                         x2T=np.ascontiguousarray(x2Tb),
                         xT=np.ascontiguousarray(xTb))
            else:
                m = dict(x1T=np.roll(x1Tb, -NQ, axis=1),
                         x2T=np.roll(x2Tb, -NQ, axis=1),
                         xT=np.roll(xTb, -NQ, axis=1))
            m.update(shared)
            in_maps.append(m)

    nc = _get_nc()
    res = run_bass_kernel_spmd(nc, in_maps, list(range(8)), trace=trace)

    out = np.empty((4, C, 64, 64), dtype=f32)
    for b in range(4):
        full = np.empty((C, N), dtype=f32)
        full[:, 0:NQ] = res.results[2 * b]["outT"]
        full[:, NQ:N] = res.results[2 * b + 1]["outT"]
        out[b] = full.reshape(C, 64, 64)
    return out, res


# revision 26
# speedup vs baseline: 1.5577x; 1.5577x over previous
"""Cross_Atten_Lite_split Trainium2 Bass kernel.

Sharding: 8 cores = (batch b in 0..3) x (query-half qh in 0..1).
Each core computes both attention heads for 2048 queries x 4096 keys of
its batch. No collectives. Math rewrites (validated vs reference):
  - eval-mode BN on x1/x2 folded into kq1_w/kq2_w (+bias).
  - channel_shuffle is a permutation of the shared q/k contraction axis
    -> eliminated;  k_h = [kq1[:,64h:64h+32]; kq2[:,64h:64h+32]],
    q_h likewise from rows 64h+32:64h+64.
  - K bias cancels in softmax (adds a per-query-row constant); dropped.
  - final BN + w_scale folded into out_w/out_b.
  - softmax without max-subtraction (max |score| ~ 67.5 < 88, fp32 safe).
  - softmax denominator via ones-augmented V (row 64 of PV output).
Matmuls run as float32r (fp32 bits, full-speed PE path at N=512).

Built on bacc.Bacc + nc.compile(): generate_event_semaphores splits
multi-wait instructions to satisfy the TRN2 1-wait-per-instruction
constraint.
"""

import numpy as np
from contextlib import ExitStack

import concourse.bass as bass
import concourse.bacc as bacc
import concourse.mybir as mybir
import concourse.tile as tile
from concourse.bass_utils import run_bass_kernel_spmd
from concourse.masks import make_identity

F32 = mybir.dt.float32
F32R = mybir.dt.float32r
AF = mybir.ActivationFunctionType

C = 256          # channels (INC1 == INC2)
N = 4096         # tokens per batch (64*64)
NQ = 2048        # queries per core
NT = 512         # free-dim tile size


def build_bass():
    nc = bacc.Bacc("TRN2", target_bir_lowering=False, debug=False, num_devices=8)

    x1T = nc.dram_tensor("x1T", [C, N], F32R, kind="ExternalInput").ap()
    x2T = nc.dram_tensor("x2T", [C, N], F32R, kind="ExternalInput").ap()
    xT = nc.dram_tensor("xT", [C, N], F32R, kind="ExternalInput").ap()
    wkq1 = nc.dram_tensor("wkq1", [2, 128, 128], F32R, kind="ExternalInput").ap()
    wkq2 = nc.dram_tensor("wkq2", [2, 128, 128], F32R, kind="ExternalInput").ap()
    wv = nc.dram_tensor("wv", [2, 128, 128], F32R, kind="ExternalInput").ap()
    wout = nc.dram_tensor("wout", [128, 256], F32R, kind="ExternalInput").ap()
    bq = nc.dram_tensor("bq", [128, 1], F32, kind="ExternalInput").ap()
    bv = nc.dram_tensor("bv", [128, 1], F32, kind="ExternalInput").ap()
    bout = nc.dram_tensor("bout", [2, 128, 1], F32, kind="ExternalInput").ap()
    onesd = nc.dram_tensor("onesd", [128, 64], F32R, kind="ExternalInput").ap()
    outT = nc.dram_tensor("outT", [C, NQ], F32, kind="ExternalOutput").ap()

    with ExitStack() as ctx:
        tc = ctx.enter_context(tile.TileContext(nc))
        const = ctx.enter_context(tc.tile_pool(name="const", bufs=1))
        pers = ctx.enter_context(tc.tile_pool(name="pers", bufs=1))

        # constants
        w_kq1 = [const.tile([128, 128], F32R, name=f"wkq1_{g}") for g in range(2)]
        w_kq2 = [const.tile([128, 128], F32R, name=f"wkq2_{g}") for g in range(2)]
        w_v = [const.tile([128, 128], F32R, name=f"wv_{g}") for g in range(2)]
        w_out = const.tile([128, 256], F32R, name="wout")
        b_q = const.tile([128, 1], F32, name="bq")
        b_v = const.tile([128, 1], F32, name="bv")
        b_out = [const.tile([128, 1], F32, name=f"bout_{g}") for g in range(2)]
        ident = const.tile([128, 128], F32, name="ident")
        ones1 = const.tile([1, 64], F32R, name="ones1")



        for g in range(2):
            nc.sync.dma_start(out=w_kq1[g][:], in_=wkq1[g])
            nc.sync.dma_start(out=w_kq2[g][:], in_=wkq2[g])
            nc.sync.dma_start(out=w_v[g][:], in_=wv[g])
            nc.sync.dma_start(out=b_out[g][:], in_=bout[g])
        nc.sync.dma_start(out=w_out[:], in_=wout[:])
        nc.sync.dma_start(out=b_q[:], in_=bq[:])
        nc.sync.dma_start(out=b_v[:], in_=bv[:])
        make_identity(nc, ident[:])
        nc.sync.dma_start(out=ones1[:], in_=onesd[0:1, 0:64])

        # persistent SBUF
        KT = pers.tile([128, N], F32R, name="KT")      # rows k1a,k1b,k2a,k2b
        QT = pers.tile([128, NQ], F32R, name="QT")     # rows q1a,q1b,q2a,q2b
        Vtok = pers.tile([128, 32 * 130], F32R, name="Vtok")
        xsb = [pers.tile([128, N], F32R, name=f"xsb_{g}") for g in range(2)]
        x1sb = [pers.tile([128, N], F32R, name=f"x1sb_{g}") for g in range(2)]
        x2sb = [pers.tile([128, N], F32R, name=f"x2sb_{g}") for g in range(2)]
        Ocat = pers.tile([128, NQ], F32R, name="Ocat")

        # fill the two ones-columns of each Vtok m-block via strided DMA
        vtok3 = Vtok.rearrange("p (m c) -> p m c", c=130)
        nc.sync.dma_start(out=vtok3[:, :, 64:65], in_=onesd[:, 0:32].rearrange("p (m c) -> p m c", c=1))
        nc.sync.dma_start(out=vtok3[:, :, 129:130], in_=onesd[:, 32:64].rearrange("p (m c) -> p m c", c=1))
        # DVE pre-touch of bias consts so later DVE ops don't wait on DMA queues
        btch = const.tile([128, 4], F32, name="btch")
        nc.vector.tensor_copy(btch[:, 0:1], b_v[:])
        nc.vector.tensor_copy(btch[:, 1:2], b_q[:])
        nc.vector.tensor_copy(btch[:, 2:3], b_out[0][:])
        nc.vector.tensor_copy(btch[:, 3:4], b_out[1][:])
        for g in range(2):
            for t in range(8):
                cs = slice(t * NT, (t + 1) * NT)
                rs = slice(128 * g, 128 * (g + 1))
                nc.sync.dma_start(out=xsb[g][:, cs], in_=xT[rs, cs])
                nc.sync.dma_start(out=x1sb[g][:, cs], in_=x1T[rs, cs])
                nc.sync.dma_start(out=x2sb[g][:, cs], in_=x2T[rs, cs])

        poolE = ctx.enter_context(tc.tile_pool(name="poolE", bufs=2))
        small = ctx.enter_context(tc.tile_pool(name="small", bufs=2))
        pout = ctx.enter_context(tc.tile_pool(name="pout", bufs=8))
        # ---------- Phase A: projections ----------
        with ExitStack() as actx:
            pvt = actx.enter_context(tc.tile_pool(name="pvt", bufs=2))
            poolA = actx.enter_context(tc.tile_pool(name="poolA", bufs=3, space="PSUM"))
            poolT = actx.enter_context(tc.tile_pool(name="poolT", bufs=4, space="PSUM"))

            for t in range(8):
                cs = slice(t * NT, (t + 1) * NT)
                if t >= 1:
                    # DVE sem is monotone: joining on iter t-1's last DVE
                    # write covers every DVE dep from iters <= t-1
                    mprev = 4 * (t - 1) + 3
                kq1p = poolA.tile([128, NT], F32, tag="mmA", name=f"kq1p_{t}")
                nc.tensor.matmul(kq1p[:], w_kq1[0][:], x1sb[0][:, cs], start=True, stop=False)
                nc.tensor.matmul(kq1p[:], w_kq1[1][:], x1sb[1][:, cs], start=False, stop=True)
                kq2p = poolA.tile([128, NT], F32, tag="mmA", name=f"kq2p_{t}")
                nc.tensor.matmul(kq2p[:], w_kq2[0][:], x2sb[0][:, cs], start=True, stop=False)
                nc.tensor.matmul(kq2p[:], w_kq2[1][:], x2sb[1][:, cs], start=False, stop=True)
                vp = poolA.tile([128, NT], F32, tag="mmA", name=f"vp_{t}")
                nc.tensor.matmul(vp[:], w_v[0][:], xsb[0][:, cs], start=True, stop=False)
                nc.tensor.matmul(vp[:], w_v[1][:], xsb[1][:, cs], start=False, stop=True)

                # scatter K/Q rows straight from PSUM (DVE only -> single sem);
                # Q bias applied during the scatter (tensor_scalar_add)
                nc.vector.tensor_copy(KT[0:32, cs], kq1p[0:32, :])
                nc.vector.tensor_copy(KT[32:64, cs], kq2p[0:32, :])
                nc.vector.tensor_copy(KT[64:96, cs], kq1p[64:96, :])
                nc.vector.tensor_copy(KT[96:128, cs], kq2p[64:96, :])
                if t < 4:  # query half
                    nc.scalar.activation(QT[0:32, cs], kq1p[32:64, :], AF.Identity, bias=b_q[0:32, :])
                    nc.scalar.activation(QT[32:64, cs], kq2p[32:64, :], AF.Identity, bias=b_q[32:64, :])
                    nc.scalar.activation(QT[64:96, cs], kq1p[96:128, :], AF.Identity, bias=b_q[64:96, :])
                    nc.scalar.activation(QT[96:128, cs], kq2p[96:128, :], AF.Identity, bias=b_q[96:128, :])
                VT = pvt.tile([128, NT], F32, tag="VT", name=f"VT_{t}")
                nc.scalar.activation(VT[:], vp[:], AF.Identity, bias=b_v[:])

                # transpose V for PV matmuls: Vtok[m] cols 0:64 = v1, 65:129 = v2
                for s in range(4):
                    m = 4 * t + s
                    ms = slice(s * 128, (s + 1) * 128)
                    tp = poolT.tile([128, 128], F32, tag="tp", name=f"tp_{m}")
                    nc.tensor.transpose(tp[:], VT[:, ms], ident[:])
                    nc.vector.tensor_copy(Vtok[:, m * 130:m * 130 + 64], tp[:, 0:64])
                    nc.vector.tensor_copy(Vtok[:, m * 130 + 65:m * 130 + 129], tp[:, 64:128])


        # ---------- Phase B: attention ----------
        with ExitStack() as bctx:
            poolS = bctx.enter_context(tc.tile_pool(name="poolS", bufs=2, space="PSUM"))
            poolO = bctx.enter_context(tc.tile_pool(name="poolO", bufs=1, space="PSUM"))
            poolCp = bctx.enter_context(tc.tile_pool(name="poolCp", bufs=1, space="PSUM"))

            for h in range(2):
                hs = slice(64 * h, 64 * (h + 1))
                for j in range(4):
                    qs = slice(j * NT, (j + 1) * NT)
                    op = poolO.tile([65, NT], F32, tag="op", name=f"op_{h}_{j}")
                    mstart = 0
                    blk = 0
                    while mstart < 32:
                        mk = min(3, 32 - mstart)
                        sp = poolS.tile([128, 3 * NT], F32, tag="sp", name=f"sp_{h}_{j}_{blk}")
                        for b4 in range(mk):
                            m = mstart + b4
                            nc.tensor.matmul(
                                sp[:, b4 * NT:(b4 + 1) * NT],
                                KT[hs, m * 128:(m + 1) * 128],
                                QT[hs, qs],
                                start=True, stop=True)
                        et = poolE.tile([128, 3 * NT], F32R, tag="et", name=f"et_{h}_{j}_{blk}")
                        nc.scalar.activation(et[:, 0:mk * NT], sp[:, 0:mk * NT],
                                             AF.Exp, scale=0.125)
                        for b4 in range(mk):
                            m = mstart + b4
                            nc.tensor.matmul(
                                op[:],
                                Vtok[:, m * 130 + 65 * h:m * 130 + 65 * h + 65],
                                et[:, b4 * NT:(b4 + 1) * NT],
                                start=(m == 0),
                                stop=(m == 31))
                        mstart += mk
                        blk += 1

                    # normalize: o[d, n] * (1 / denom[n]); denom = op[64, :]
                    rec = small.tile([1, NT], F32R, tag="rec", name=f"rec_{h}_{j}")
                    with nc.allow_low_precision(reason="f32r is fp32 bits"):
                        nc.vector.reciprocal(rec[:], op[64:65, :])
                    rb = poolCp.tile([64, NT], F32, tag="rb", name=f"rb_{h}_{j}")
                    nc.tensor.matmul(rb[:], ones1[:], rec[:], start=True, stop=True)
                    rbs = small.tile([64, NT], F32, tag="rbs", name=f"rbs_{h}_{j}")
                    nc.vector.tensor_copy(rbs[:], rb[:])
                    nc.vector.tensor_mul(Ocat[hs, qs], op[0:64, :], rbs[:])

            # ---------- Phase C: output projection + bias + residual ----------
            for j in range(4):
                qs = slice(j * NT, (j + 1) * NT)
                for g in range(2):
                    pp = poolCp.tile([128, NT], F32, tag="rb", name=f"pp_{j}_{g}")
                    nc.tensor.matmul(pp[:], w_out[:, 128 * g:128 * (g + 1)],
                                     Ocat[:, qs], start=True, stop=True)
                    osb = pout.tile([128, NT], F32, tag="osb", name=f"osb_{j}_{g}")
                    nc.vector.scalar_tensor_tensor(
                        osb[:], pp[:], b_out[g][:], xsb[g][:, qs].bitcast(F32),
                        op0=bass.mybir.AluOpType.add, op1=bass.mybir.AluOpType.add)
                    nc.sync.dma_start(out=outT[128 * g:128 * (g + 1), qs], in_=osb[:])

    nc.compile()
    return nc


_NC = None


def _get_nc():
    global _NC
    if _NC is None:
        _NC = build_bass()
    return _NC


def kernel(**inputs):
    out, _ = _run(inputs, trace=False)
    return out


def _run(inputs, trace=False):
    eps = 1e-5
    f32 = np.float32
    inp = {k: np.asarray(v, dtype=np.float32) for k, v in inputs.items()}

    s1 = inp['bn1_g'] / np.sqrt(inp['bn1_v'] + eps)
    t1 = inp['bn1_b'] - inp['bn1_m'] * s1
    s2 = inp['bn2_g'] / np.sqrt(inp['bn2_v'] + eps)
    t2 = inp['bn2_b'] - inp['bn2_m'] * s2
    W1 = inp['kq1_w'] * s1[None, :]
    b1 = inp['kq1_b'] + inp['kq1_w'] @ t1
    W2 = inp['kq2_w'] * s2[None, :]
    b2 = inp['kq2_b'] + inp['kq2_w'] @ t2
    sl = inp['bnl_g'] / np.sqrt(inp['bnl_v'] + eps)
    tl = inp['bnl_b'] - inp['bnl_m'] * sl
    ws = inp['w_scale'][0]
    Wout = (ws * sl)[:, None] * inp['out_w']
    bout_f = ws * (sl * inp['out_b'] + tl)

    wkq1 = np.ascontiguousarray(W1.T.reshape(2, 128, 128), dtype=f32)
    wkq2 = np.ascontiguousarray(W2.T.reshape(2, 128, 128), dtype=f32)
    wv = np.ascontiguousarray(inp['v_w'].T.reshape(2, 128, 128), dtype=f32)
    wout_a = np.ascontiguousarray(Wout.T, dtype=f32)
    bq = np.concatenate([b1[32:64], b2[32:64], b1[96:128], b2[96:128]]
                        ).reshape(128, 1).astype(f32)
    bv = inp['v_b'].reshape(128, 1).astype(f32)
    bout_a = bout_f.reshape(2, 128, 1).astype(f32)

    shared = dict(wkq1=wkq1, wkq2=wkq2, wv=wv, wout=wout_a, bq=bq, bv=bv,
                  bout=bout_a, onesd=np.ones((128, 64), dtype=f32))

    in_maps = []
    for b in range(4):
        x1Tb = inp['x1'][b].reshape(C, N)
        x2Tb = inp['x2'][b].reshape(C, N)
        xTb = inp['x'][b].reshape(C, N)
        for qh in range(2):
            if qh == 0:
                m = dict(x1T=np.ascontiguousarray(x1Tb),
                         x2T=np.ascontiguousarray(x2Tb),
                         xT=np.ascontiguousarray(xTb))
            else:
                m = dict(x1T=np.roll(x1Tb, -NQ, axis=1),
                         x2T=np.roll(x2Tb, -NQ, axis=1),
                         xT=np.roll(xTb, -NQ, axis=1))
            m.update(shared)
            in_maps.append(m)

    nc = _get_nc()
    res = run_bass_kernel_spmd(nc, in_maps, list(range(8)), trace=trace)

    out = np.empty((4, C, 64, 64), dtype=f32)
    for b in range(4):
        full = np.empty((C, N), dtype=f32)
        full[:, 0:NQ] = res.results[2 * b]["outT"]
        full[:, NQ:N] = res.results[2 * b + 1]["outT"]
        out[b] = full.reshape(C, 64, 64)
    return out, res
